# revision 23
# baseline (speedup 1.0000x reference)
"""Trainium2 Bass kernel for nn_Attn2d (3x3 local window attention, 8 heads).

Sharding: 8 cores = (batch 4) x (H halves 2). Each core gets a halo-extended
slice of x (34 rows incl 1-row halo each side, zero-filled outside the image),
computes the 1x1 conv projection + windowed attention for its 32 own rows.

Design: f32r projection (psum-exact); q/k bf16 and v f32r in row-interleaved
[r][t][col] merged tiles (tight subtile deps let attention chunks start as
soon as their rows are projected). Per 256-px chunk: logits via per-dl q*k
pair multiplies (DVE 2x / Pool) + bf16 head-reduce matmuls into [72,256]
psum; softmax (ACT exp -> sum9 matmul -> DVE reciprocal -> e8 expand ->
normalize, all f32r); AV: expand attn->channel space (f32r matmuls, both
dl of a pair into one 2-bank psum tile), p2 = E*v_shift (DVE direct-psum /
ACT-cast + Pool), then t0 taps accumulate on PE (identity matmuls into one
psum bank) while t1 taps accumulate on Pool (SBUF f32r adds) to split the
accumulation load. Emission is software-pipelined via generator zippers:
proj groups, logits(ci) and AV(ci-1) interleave in every engine FIFO.
"""
import numpy as np

import bass_rust
import concourse.mybir as mybir
import concourse.tile as tile
from concourse import bacc

F32 = mybir.dt.float32
F32R = mybir.dt.float32r
BF16 = mybir.dt.bfloat16
F8 = mybir.dt.float8e4
AF = mybir.ActivationFunctionType
DR = mybir.MatmulPerfMode.DoubleRow
MUL = mybir.AluOpType.mult

# problem constants (hardcoded per contract)
B, CIN, H, W = 4, 256, 64, 64
QK = 256
OUT = 256
NH = 8
KW = 3
D = QK // NH          # 32
SCALE = float(D) ** (-0.25)
NCORES = 8

HOWN = H // 2         # 32 own rows per core
HS = HOWN + 2         # 34 rows incl halo
WP = W + 4            # 68 padded width (interior cols 2..65)
C0 = 2                # first interior column
WP2 = 2 * WP          # row stride: both t-halves interleaved per row
PXP = HS * WP         # padded pixels per t-half
PXU = HS * W          # 2176 unpadded input pixels
OWNPX = HOWN * W      # 2048 own pixels
NKK = KW * KW         # 9
NL = NH * NKK         # 72 (n, delta) pairs

RC = 4                # rows per attention chunk
CHUNK = RC * W        # 256 px per chunk
NCHUNK = OWNPX // CHUNK


def _view(tl, offset, dims):
    """AP over tile `tl` with free dims [(stride, count), ...] at offset."""
    ap = tl[:].copy()
    ap.ap = bass_rust.VecI64Pair([list(tl[:].ap[0])] + [[s, n] for s, n in dims])
    ap.offset = offset
    return ap


def _off(ci, dl, t):
    di, dj = dl // KW, dl % KW
    return (RC * ci + di) * WP2 + t * WP + C0 - 1 + dj


def _build_nc():
    nc = bacc.Bacc()

    xin = nc.declare_dram_parameter("x", [CIN, PXU], F32R, isOutput=False)
    wt = nc.declare_dram_parameter("wt", [CIN, 3 * QK], F32R, isOutput=False)
    biasd = nc.declare_dram_parameter("bias", [128, 6], F32, isOutput=False)
    posd = nc.declare_dram_parameter("posblk", [CIN, NL], BF16, isOutput=False)
    redd = nc.declare_dram_parameter("redpair", [128, NKK * 2 * NL], BF16, isOutput=False)
    sum9d = nc.declare_dram_parameter("sum9", [NL, NH], F32R, isOutput=False)
    e8d = nc.declare_dram_parameter("e8", [NH, NL], BF16, isOutput=False)
    expd = nc.declare_dram_parameter("expall", [NL, NKK * CIN], F32R, isOutput=False)
    identd = nc.declare_dram_parameter("identpair", [128, 128], F32R, isOutput=False)
    edged = nc.declare_dram_parameter("edge", [128, 2], F32, isOutput=False)
    outd = nc.declare_dram_parameter("o", [OUT, OWNPX], F32, isOutput=True)

    with tile.TileContext(nc) as tc:
        with (
            tc.tile_pool(name="const", bufs=1) as constp,
            tc.tile_pool(name="data", bufs=1) as datap,
            tc.tile_pool(name="work", bufs=4) as workp,
        ):
            # ---- load inputs ----
            x_t = [datap.tile([128, PXU], F32R, tag=f"x{t}", name=f"x{t}") for t in range(2)]
            wt_t = [datap.tile([128, 3 * QK], F32R, tag=f"wt{t}", name=f"wt{t}") for t in range(2)]
            XQ = PXU // 4
            for t in range(2):
                nc.sync.dma_start(wt_t[t][:], wt[t * 128:(t + 1) * 128, :])
            for qi in range(4):
                for t in range(2):
                    eng = nc.sync if (qi + t) % 2 == 0 else nc.gpsimd
                    eng.dma_start(x_t[t][:, qi * XQ:(qi + 1) * XQ],
                                  xin[t * 128:(t + 1) * 128, qi * XQ:(qi + 1) * XQ])
            bias_t = constp.tile([128, 6], F32, tag="bias", name="bias")
            nc.gpsimd.dma_start(bias_t[:], biasd[:])
            edge_t = constp.tile([128, 2], F32, tag="edge", name="edge")
            nc.gpsimd.dma_start(edge_t[:], edged[:])
            pos_r = [constp.tile([128, NL], BF16, tag=f"posr{t}", name=f"posr{t}") for t in range(2)]
            for t in range(2):
                nc.gpsimd.dma_start(pos_r[t][:], posd[t * 128:(t + 1) * 128, :])
            red_r = constp.tile([128, NKK * 2 * NL], BF16, tag="redr", name="redr")
            nc.gpsimd.dma_start(red_r[:], redd[:])
            # f32r 0/1 stationaries DMA'd directly (f32r moving data requires
            # matching 32-bit weights; f32r is bit-identical to host f32)
            sum9_r2 = constp.tile([NL, NH], F32R, tag="sum9r2", name="sum9r2")
            nc.gpsimd.dma_start(sum9_r2[:], sum9d[:])
            e8_r = constp.tile([NH, NL], BF16, tag="e8r", name="e8r")
            nc.gpsimd.dma_start(e8_r[:], e8d[:])
            exp_r2 = constp.tile([NL, NKK * CIN], F32R, tag="expallr2", name="expallr2")
            nc.gpsimd.dma_start(exp_r2[:], expd[:])
            ident_r2 = constp.tile([128, 128], F32R, tag="identr2", name="identr2")
            nc.gpsimd.dma_start(ident_r2[:], identd[:])

            # warm the ACT exp table while waiting for x
            warm = workp.tile([128, 1], F32, tag="warm", name="warm")
            nc.scalar.activation(warm[:], bias_t[:, 0:1], AF.Exp)

            # ---- q/k/v merged bf16 tiles, row layout [r][t][col] so a
            # chunk's views only span its own rows (tight subtile deps) ----
            q_b = datap.tile([128, HS * WP2], BF16, tag="qb", name="qb")
            k_b = datap.tile([128, HS * WP2], BF16, tag="kb", name="kb")
            v_b = datap.tile([128, HS * WP2], F32R, tag="vb", name="vb")
            # zero only the pad columns (proj overwrites interior after);
            # bf16 tiles via f32-bitcast (col j covers bf16 cols 2j,2j+1)
            for tl in (q_b, k_b):
                fv = tl[:].bitcast(F32).rearrange("p (r c) -> p r c", c=WP)
                nc.gpsimd.memset(fv[:, :, 0:1], 0.0)
                nc.gpsimd.memset(fv[:, :, WP // 2 - 1:WP // 2 + 1], 0.0)
                nc.gpsimd.memset(fv[:, :, WP - 1:WP], 0.0)
            fv = v_b[:].bitcast(F32).rearrange("p (r c) -> p r c", c=WP2)
            nc.gpsimd.memset(fv[:, :, 0:C0], 0.0)
            nc.gpsimd.memset(fv[:, :, WP - C0:WP + C0], 0.0)
            nc.gpsimd.memset(fv[:, :, WP2 - C0:WP2], 0.0)

            # ---- projection: bf16 matmuls, contraction 256 via 2 psum passes,
            # ci-major so chunk 0 attention can start early ----
            pxc = [448, 448, 448, 448, 384]
            pxo = [0, 448, 896, 1344, 1792]

            def pad_view(tl, t, r0, nr, c0, cw=W):
                return _view(tl, r0 * WP2 + t * WP + c0, [(WP2, nr), (1, cw)])

            ps_stack = tc.tile_pool(name="psl", bufs=1, space="PSUM")
            psl = ps_stack.__enter__()
            pse_stack = tc.tile_pool(name="pse", bufs=1, space="PSUM")
            pse = pse_stack.__enter__()
            pso_stack = tc.tile_pool(name="pso", bufs=1, space="PSUM")
            pso = pso_stack.__enter__()

            def emit_proj(cis):
                for ci in cis:
                    cw, co = pxc[ci], pxo[ci]
                    r0, nr = co // W, cw // W
                    for m in range(6):
                        grp = m // 2        # 0=q, 1=k, 2=v
                        t = m % 2
                        dst = (q_b, k_b, v_b)[grp]
                        ppb = pse.tile([128, 2 * CHUNK], F32, tag="pe", name="pp", bufs=3)
                        pp = ppb[:, :cw]
                        for kt in range(2):
                            nc.tensor.matmul(
                                pp[:],
                                wt_t[kt][:, m * 128:(m + 1) * 128],
                                x_t[kt][:, co:co + cw],
                                start=(kt == 0),
                                stop=(kt == 1),
                            )
                        if grp == 2:
                            nc.vector.tensor_scalar_add(pad_view(dst, t, r0, nr, C0), pp[:],
                                                        bias_t[:, m:m + 1])
                        else:
                            nc.scalar.activation(pad_view(dst, t, r0, nr, C0), pp[:],
                                                 AF.Identity, bias=bias_t[:, m:m + 1])

            def emit_edge(row, col):
                # zero k/v halo row that falls outside the image
                for tl in (k_b, v_b):
                    rv = _view(tl, row * WP2, [(1, WP2)])
                    nc.gpsimd.tensor_scalar_mul(rv, rv, edge_t[:, col:col + 1])

            def gen_logits(ci, out):
                # pl [72,512] = pos-term + per-dl fp8 DoubleRow reductions
                pl = psl.tile([NL, CHUNK], F32, tag="pl", name="pl")
                for t in range(2):
                    nc.tensor.matmul(
                        pl[:], pos_r[t][:], pad_view(q_b, t, 1 + RC * ci, RC, C0),
                        start=(t == 0), stop=False, skip_group_check=True,
                    )
                qv = _view(q_b, (RC * ci + 1) * WP2 + C0,
                           [(WP, 2), (WP2, RC), (1, W)])
                for dl in range(NKK):
                    kv = _view(k_b, _off(ci, dl, 0), [(WP, 2), (WP2, RC), (1, W)])
                    pr = workp.tile([128, 2 * CHUNK], BF16, tag="pr", name=f"pr{dl}", bufs=6)
                    prw = pr[:].rearrange("p (i r c) -> p i r c", i=2, c=W)
                    eng = nc.vector if dl % 2 == 0 else nc.gpsimd
                    eng.tensor_mul(prw, qv, kv)
                    for t in range(2):
                        nc.tensor.matmul(
                            pl[:],
                            _view(red_r, (2 * dl + t) * NL, [(1, NL)]),
                            pr[:, t * CHUNK:(t + 1) * CHUNK],
                            start=False, stop=(dl == NKK - 1 and t == 1),
                            skip_group_check=True,
                        )
                    yield
                e_t = workp.tile([NL, CHUNK], F32R, tag="e", name="e", bufs=3)
                nc.scalar.activation(e_t[:], pl[:], AF.Exp)
                pz = pse.tile([NH, CHUNK], F32, tag="pzx", name="pz")
                nc.tensor.matmul(pz[:], sum9_r2[:], e_t[:],
                                 start=True, stop=True, skip_group_check=True)
                rz8 = workp.tile([NH, CHUNK], BF16, tag="rz8", name="rz8")
                with nc.allow_low_precision(reason="bf16 softmax denominators"):
                    nc.vector.reciprocal(rz8[:], pz[:])
                yield
                pzr = pse.tile([NL, CHUNK], F32, tag="pzx", name="pzr")
                nc.tensor.matmul(pzr[:], e8_r[:], rz8[:],
                                 start=True, stop=True, skip_group_check=True)
                attn = workp.tile([NL, CHUNK], F32R, tag="attn", name="attn", bufs=3)
                nc.vector.tensor_mul(attn[:], e_t[:], pzr[:].bitcast(F32R))
                out["attn"] = attn

            def gen_av(ci, attn):
                # t0 taps accumulate on PE into one psum bank; t1 taps
                # accumulate on Pool into an SBUF f32r accumulator (PE relief)
                po = pso.tile([128, CHUNK], F32, tag="po", name="po")
                po1 = None
                if ci == NCHUNK - 1:
                    po1 = pso.tile([128, CHUNK], F32, tag="po1", name="po1")
                acc1 = workp.tile([128, CHUNK], F32R, tag="acc1", name="acc1", bufs=2)
                pend = []

                def flush():
                    while pend:
                        pend.pop(0)()

                for j in range(4):
                    dl0 = 2 * j
                    for t in range(2):
                        pe = pse.tile([128, 2 * CHUNK], F32, tag="pe", name="pe", bufs=3)
                        for i in range(2):
                            nc.tensor.matmul(
                                pe[:, i * CHUNK:(i + 1) * CHUNK],
                                exp_r2[:, (dl0 + i) * CIN + t * 128:
                                      (dl0 + i) * CIN + (t + 1) * 128],
                                attn[:], start=True, stop=True, skip_group_check=True,
                            )
                        o0 = _off(ci, dl0, t)
                        vv = _view(v_b, o0, [(_off(ci, dl0 + 1, t) - o0, 2), (WP2, RC), (1, W)])
                        p2 = workp.tile([128, 2 * CHUNK], F32R, tag="p2", name=f"p2_{t}", bufs=6)
                        p2w = p2[:].rearrange("p (i r c) -> p i r c", i=2, c=W)
                        pew = pe[:].rearrange("p (i r c) -> p i r c", i=2, c=W)
                        if t == 0:
                            if j < 3:
                                # DVE reads psum directly (1x)
                                nc.vector.tensor_mul(p2w, pew.bitcast(F32R), vv)
                            else:
                                eb = workp.tile([128, 2 * CHUNK], F32R, tag="eb", name="eb0", bufs=3)
                                nc.scalar.copy(eb[:], pe[:])
                                ebw = eb[:].rearrange("p (i r c) -> p i r c", i=2, c=W)
                                nc.gpsimd.tensor_mul(p2w, ebw, vv)

                            def acc(p2=p2, j=j):
                                for i in range(2):
                                    nc.tensor.matmul(
                                        po[:], ident_r2[:],
                                        p2[:, i * CHUNK:(i + 1) * CHUNK],
                                        start=(j == 0 and i == 0), stop=False,
                                        skip_group_check=True,
                                    )
                            pend.append(acc)
                        else:
                            # ACT casts psum->f32r, Pool multiplies + accumulates
                            eb = workp.tile([128, 2 * CHUNK], F32R, tag="eb", name="eb1", bufs=3)
                            nc.scalar.copy(eb[:], pe[:])
                            ebw = eb[:].rearrange("p (i r c) -> p i r c", i=2, c=W)
                            nc.gpsimd.tensor_mul(p2w, ebw, vv)

                            if ci == NCHUNK - 1:
                                # drain: PE is idle, accumulate there instead
                                def acc(p2=p2, j=j):
                                    for i in range(2):
                                        nc.tensor.matmul(
                                            po1[:], ident_r2[:],
                                            p2[:, i * CHUNK:(i + 1) * CHUNK],
                                            start=(j == 0 and i == 0), stop=False,
                                            skip_group_check=True,
                                        )
                            else:
                                def acc(p2=p2, j=j):
                                    if j == 0:
                                        nc.gpsimd.tensor_add(acc1[:], p2[:, :CHUNK], p2[:, CHUNK:])
                                    else:
                                        nc.gpsimd.tensor_add(acc1[:], acc1[:], p2[:, :CHUNK])
                                        nc.gpsimd.tensor_add(acc1[:], acc1[:], p2[:, CHUNK:])
                            pend.append(acc)
                        yield
                        if len(pend) > 1:
                            pend.pop(0)()
                for t in range(2):
                    peb = pse.tile([128, 2 * CHUNK], F32, tag="pe", name="pes", bufs=3)
                    pe = peb[:, :CHUNK]
                    nc.tensor.matmul(
                        pe[:], exp_r2[:, 8 * CIN + t * 128: 8 * CIN + (t + 1) * 128],
                        attn[:], start=True, stop=True, skip_group_check=True,
                    )
                    vv = _view(v_b, _off(ci, 8, t), [(WP2, RC), (1, W)])
                    p2 = workp.tile([128, CHUNK], F32R, tag="p2s", name=f"p2s{t}", bufs=2)
                    p2w = p2[:].rearrange("p (r c) -> p r c", c=W)
                    pew = pe[:].rearrange("p (r c) -> p r c", c=W)
                    if t == 0:
                        nc.vector.tensor_mul(p2w, pew.bitcast(F32R), vv)

                        def accs(p2=p2):
                            nc.tensor.matmul(
                                po[:], ident_r2[:],
                                p2[:], start=False, stop=True, skip_group_check=True,
                            )
                        pend.append(accs)
                    else:
                        eb = workp.tile([128, CHUNK], F32R, tag="ebs", name="ebs", bufs=2)
                        nc.scalar.copy(eb[:], pe[:])
                        nc.gpsimd.tensor_mul(p2w, eb[:].rearrange("p (r c) -> p r c", c=W), vv)

                        if ci == NCHUNK - 1:
                            def accs(p2=p2):
                                nc.tensor.matmul(
                                    po1[:], ident_r2[:],
                                    p2[:], start=False, stop=True, skip_group_check=True,
                                )
                        else:
                            def accs(p2=p2):
                                nc.gpsimd.tensor_add(acc1[:], acc1[:], p2[:])
                        pend.append(accs)
                    yield
                    if len(pend) > 1:
                        pend.pop(0)()
                flush()
                ob = workp.tile([128, CHUNK], F32, tag="ob", name="ob", bufs=2)
                nc.scalar.copy(ob[:], po[:])
                nc.sync.dma_start(outd[0:128, ci * CHUNK:(ci + 1) * CHUNK], ob[:])
                if ci == NCHUNK - 1:
                    ob1 = workp.tile([128, CHUNK], F32, tag="ob1", name="ob1")
                    nc.vector.tensor_copy(ob1[:], po1[:])
                    nc.sync.dma_start(outd[128:256, ci * CHUNK:(ci + 1) * CHUNK], ob1[:])
                else:
                    nc.sync.dma_start(outd[128:256, ci * CHUNK:(ci + 1) * CHUNK],
                                      acc1[:].bitcast(F32))

            def gen_proj(cis):
                for ci in cis:
                    cw, co = pxc[ci], pxo[ci]
                    r0, nr = co // W, cw // W
                    for m in range(6):
                        grp = m // 2        # 0=q, 1=k, 2=v
                        t = m % 2
                        dst = (q_b, k_b, v_b)[grp]
                        ppb = pse.tile([128, 2 * CHUNK], F32, tag="pe", name="pp", bufs=3)
                        pp = ppb[:, :cw]
                        for kt in range(2):
                            nc.tensor.matmul(
                                pp[:],
                                wt_t[kt][:, m * 128:(m + 1) * 128],
                                x_t[kt][:, co:co + cw],
                                start=(kt == 0),
                                stop=(kt == 1),
                            )
                        if grp == 2 and t == 0:
                            nc.vector.tensor_scalar_add(pad_view(dst, t, r0, nr, C0), pp[:],
                                                        bias_t[:, m:m + 1])
                        else:
                            nc.scalar.activation(pad_view(dst, t, r0, nr, C0), pp[:],
                                                 AF.Identity, bias=bias_t[:, m:m + 1])
                        yield

            def zipper(*gens):
                live = [iter(g) for g in gens]
                while live:
                    nxt = []
                    for g in live:
                        try:
                            next(g)
                            nxt.append(g)
                        except StopIteration:
                            pass
                    live = nxt

            # software pipeline: zip logits(ci), AV(ci-1) and the next proj
            # group so every engine FIFO interleaves all concurrent stages
            hs = [{} for _ in range(NCHUNK)]
            zipper(gen_proj([0]))
            emit_edge(0, 0)
            zipper(gen_logits(0, hs[0]), gen_proj([1]))
            proj_at = {1: [2], 3: [3], 5: [4]}
            for ci in range(1, NCHUNK):
                gens = [gen_logits(ci, hs[ci]), gen_av(ci - 1, hs[ci - 1]["attn"])]
                if ci in proj_at:
                    gens.append(gen_proj(proj_at[ci]))
                if ci == 6:
                    emit_edge(HS - 1, 1)
                zipper(*gens)
            zipper(gen_av(NCHUNK - 1, hs[NCHUNK - 1]["attn"]))

            pso_stack.__exit__(None, None, None)
            pse_stack.__exit__(None, None, None)
            ps_stack.__exit__(None, None, None)

    nc.finalize()
    return nc


_CACHE = {}


def _host_consts(w_proj, b_proj, pos_feats):
    import ml_dtypes
    bf = ml_dtypes.bfloat16
    f8 = ml_dtypes.float8_e4m3

    wT = np.ascontiguousarray(w_proj.T).astype(np.float32).copy()   # [256, 768]
    wT[:, : 2 * QK] *= SCALE
    b = np.asarray(b_proj, np.float32).copy()
    b[: 2 * QK] *= SCALE
    bias = np.ascontiguousarray(b.reshape(6, 128).T)                # [128, 6]

    heads = np.arange(CIN) // D                                     # head of channel
    posblk = np.zeros((CIN, NL), np.float32)
    for c in range(CIN):
        n = heads[c]
        posblk[c, n * NKK:(n + 1) * NKK] = pos_feats[c]

    # redpair [128, 9*2*72] bf16: [k, dl, t, nl] = 1 iff nl == head(k+128t)*9+dl
    redpair = np.zeros((128, NKK, 2, NL), np.float32)
    for k in range(128):
        for t in range(2):
            n = heads[k + 128 * t]
            for dl in range(NKK):
                redpair[k, dl, t, n * NKK + dl] = 1.0

    sum9 = np.zeros((NL, NH), np.float32)
    e8 = np.zeros((NH, NL), np.float32)
    for n in range(NH):
        for dl in range(NKK):
            sum9[n * NKK + dl, n] = 1.0
            e8[n, n * NKK + dl] = 1.0

    expall = np.zeros((NL, NKK * CIN), np.float32)
    for dl in range(NKK):
        for n in range(NH):
            expall[n * NKK + dl, dl * CIN + n * D: dl * CIN + (n + 1) * D] = 1.0

    ident = np.eye(128, dtype=np.float32)

    return (wT.astype(np.float32), bias, posblk.astype(bf),
            redpair.reshape(128, NKK * 2 * NL).astype(bf),
            sum9.astype(np.float32), e8.astype(bf), expall.astype(np.float32),
            ident)


def make_in_maps(x, w_proj, b_proj, pos_feats):
    import ml_dtypes
    bf = ml_dtypes.bfloat16

    x = np.asarray(x, np.float32)
    wT, bias, posblk, redpair, sum9, e8, expall, identpair = _host_consts(
        np.asarray(w_proj, np.float32),
        np.asarray(b_proj, np.float32),
        np.asarray(pos_feats, np.float32),
    )
    in_maps = []
    for s in range(NCORES):
        b_i, half = s // 2, s % 2
        xs = np.zeros((CIN, HS, W), np.float32)
        h0 = half * HOWN - 1                  # global row of local row 0
        lo, hi = max(h0, 0), min(h0 + HS, H)
        xs[:, lo - h0:hi - h0, :] = x[b_i, :, lo:hi, :]
        edge = np.ones((128, 2), np.float32)
        if half == 0:
            edge[:, 0] = 0.0
        if half == 1:
            edge[:, 1] = 0.0
        in_maps.append({
            "x": np.ascontiguousarray(xs.reshape(CIN, PXU)),
            "wt": wT, "bias": bias, "posblk": posblk, "redpair": redpair,
            "sum9": sum9, "e8": e8, "expall": expall, "identpair": identpair,
            "edge": edge,
        })
    return in_maps


def kernel(x, w_proj, b_proj, pos_feats):
    from concourse.bass_utils import run_bass_kernel_spmd

    if "nc" not in _CACHE:
        _CACHE["nc"] = _build_nc()
    nc = _CACHE["nc"]
    in_maps = make_in_maps(x, w_proj, b_proj, pos_feats)
    res = run_bass_kernel_spmd(nc, in_maps, list(range(NCORES)))
    out = np.zeros((B, OUT, H, W), np.float32)
    for s in range(NCORES):
        b_i, half = s // 2, s % 2
        out[b_i, :, half * HOWN:(half + 1) * HOWN, :] = (
            res.results[s]["o"].reshape(OUT, HOWN, W)
        )
    return out


# revision 25
# speedup vs baseline: 1.0010x; 1.0010x over previous
"""Trainium2 Bass kernel for nn_Attn2d (3x3 local window attention, 8 heads).

Sharding: 8 cores = (batch 4) x (H halves 2). Each core gets a halo-extended
slice of x (34 rows incl 1-row halo each side, zero-filled outside the image),
computes the 1x1 conv projection + windowed attention for its 32 own rows.

Design: f32r projection (psum-exact); q/k bf16 and v f32r in row-interleaved
[r][t][col] merged tiles (tight subtile deps let attention chunks start as
soon as their rows are projected). Per 256-px chunk: logits via per-dl q*k
pair multiplies (DVE 2x / Pool) + bf16 head-reduce matmuls into [72,256]
psum; softmax (ACT exp -> sum9 matmul -> DVE reciprocal -> e8 expand ->
normalize, all f32r); AV: expand attn->channel space (f32r matmuls, both
dl of a pair into one 2-bank psum tile), p2 = E*v_shift (DVE direct-psum /
ACT-cast + Pool), then t0 taps accumulate on PE (identity matmuls into one
psum bank) while t1 taps accumulate on Pool (SBUF f32r adds) to split the
accumulation load. Emission is software-pipelined via generator zippers:
proj groups, logits(ci) and AV(ci-1) interleave in every engine FIFO.
"""
import numpy as np

import bass_rust
import concourse.mybir as mybir
import concourse.tile as tile
from concourse import bacc

F32 = mybir.dt.float32
F32R = mybir.dt.float32r
BF16 = mybir.dt.bfloat16
F8 = mybir.dt.float8e4
AF = mybir.ActivationFunctionType
DR = mybir.MatmulPerfMode.DoubleRow
MUL = mybir.AluOpType.mult

# problem constants (hardcoded per contract)
B, CIN, H, W = 4, 256, 64, 64
QK = 256
OUT = 256
NH = 8
KW = 3
D = QK // NH          # 32
SCALE = float(D) ** (-0.25)
NCORES = 8

HOWN = H // 2         # 32 own rows per core
HS = HOWN + 2         # 34 rows incl halo
WP = W + 4            # 68 padded width (interior cols 2..65)
C0 = 2                # first interior column
WP2 = 2 * WP          # row stride: both t-halves interleaved per row
PXP = HS * WP         # padded pixels per t-half
PXU = HS * W          # 2176 unpadded input pixels
OWNPX = HOWN * W      # 2048 own pixels
NKK = KW * KW         # 9
NL = NH * NKK         # 72 (n, delta) pairs

RC = 4                # rows per attention chunk
CHUNK = RC * W        # 256 px per chunk
NCHUNK = OWNPX // CHUNK


def _view(tl, offset, dims):
    """AP over tile `tl` with free dims [(stride, count), ...] at offset."""
    ap = tl[:].copy()
    ap.ap = bass_rust.VecI64Pair([list(tl[:].ap[0])] + [[s, n] for s, n in dims])
    ap.offset = offset
    return ap


def _off(ci, dl, t):
    di, dj = dl // KW, dl % KW
    return (RC * ci + di) * WP2 + t * WP + C0 - 1 + dj


def _build_nc():
    nc = bacc.Bacc()

    xin = nc.declare_dram_parameter("x", [CIN, PXU], F32R, isOutput=False)
    wt = nc.declare_dram_parameter("wt", [CIN, 3 * QK], F32R, isOutput=False)
    biasd = nc.declare_dram_parameter("bias", [128, 6], F32, isOutput=False)
    posd = nc.declare_dram_parameter("posblk", [CIN, NL], BF16, isOutput=False)
    redd = nc.declare_dram_parameter("redpair", [128, NKK * 2 * NL], BF16, isOutput=False)
    sum9d = nc.declare_dram_parameter("sum9", [NL, NH], F32R, isOutput=False)
    e8d = nc.declare_dram_parameter("e8", [NH, NL], BF16, isOutput=False)
    expd = nc.declare_dram_parameter("expall", [NL, NKK * CIN], F32R, isOutput=False)
    identd = nc.declare_dram_parameter("identpair", [128, 128], F32R, isOutput=False)
    edged = nc.declare_dram_parameter("edge", [128, 2], F32, isOutput=False)
    outd = nc.declare_dram_parameter("o", [OUT, OWNPX], F32, isOutput=True)

    with tile.TileContext(nc) as tc:
        with (
            tc.tile_pool(name="const", bufs=1) as constp,
            tc.tile_pool(name="data", bufs=1) as datap,
            tc.tile_pool(name="work", bufs=4) as workp,
        ):
            # ---- load inputs ----
            x_t = [datap.tile([128, PXU], F32R, tag=f"x{t}", name=f"x{t}") for t in range(2)]
            wt_t = [datap.tile([128, 3 * QK], F32R, tag=f"wt{t}", name=f"wt{t}") for t in range(2)]
            XQ = PXU // 4
            for t in range(2):
                nc.sync.dma_start(wt_t[t][:], wt[t * 128:(t + 1) * 128, :])
            for qi in range(4):
                for t in range(2):
                    eng = nc.sync if (qi + t) % 2 == 0 else nc.gpsimd
                    eng.dma_start(x_t[t][:, qi * XQ:(qi + 1) * XQ],
                                  xin[t * 128:(t + 1) * 128, qi * XQ:(qi + 1) * XQ])
            bias_t = constp.tile([128, 6], F32, tag="bias", name="bias")
            nc.gpsimd.dma_start(bias_t[:], biasd[:])
            edge_t = constp.tile([128, 2], F32, tag="edge", name="edge")
            nc.gpsimd.dma_start(edge_t[:], edged[:])
            pos_r = [constp.tile([128, NL], BF16, tag=f"posr{t}", name=f"posr{t}") for t in range(2)]
            for t in range(2):
                nc.gpsimd.dma_start(pos_r[t][:], posd[t * 128:(t + 1) * 128, :])
            red_r = constp.tile([128, NKK * 2 * NL], BF16, tag="redr", name="redr")
            nc.gpsimd.dma_start(red_r[:], redd[:])
            # f32r 0/1 stationaries DMA'd directly (f32r moving data requires
            # matching 32-bit weights; f32r is bit-identical to host f32)
            sum9_r2 = constp.tile([NL, NH], F32R, tag="sum9r2", name="sum9r2")
            nc.gpsimd.dma_start(sum9_r2[:], sum9d[:])
            e8_r = constp.tile([NH, NL], BF16, tag="e8r", name="e8r")
            nc.gpsimd.dma_start(e8_r[:], e8d[:])
            exp_r2 = constp.tile([NL, NKK * CIN], F32R, tag="expallr2", name="expallr2")
            nc.gpsimd.dma_start(exp_r2[:], expd[:])
            ident_r2 = constp.tile([128, 128], F32R, tag="identr2", name="identr2")
            nc.gpsimd.dma_start(ident_r2[:], identd[:])

            # warm the ACT exp table while waiting for x
            warm = workp.tile([128, 1], F32, tag="warm", name="warm")
            nc.scalar.activation(warm[:], bias_t[:, 0:1], AF.Exp)

            # ---- q/k/v merged bf16 tiles, row layout [r][t][col] so a
            # chunk's views only span its own rows (tight subtile deps) ----
            q_b = datap.tile([128, HS * WP2], BF16, tag="qb", name="qb")
            k_b = datap.tile([128, HS * WP2], BF16, tag="kb", name="kb")
            v_b = datap.tile([128, HS * WP2], F32R, tag="vb", name="vb")
            # zero only the pad columns (proj overwrites interior after);
            # bf16 tiles via f32-bitcast (col j covers bf16 cols 2j,2j+1)
            for tl in (q_b, k_b):
                fv = tl[:].bitcast(F32).rearrange("p (r c) -> p r c", c=WP)
                nc.gpsimd.memset(fv[:, :, 0:1], 0.0)
                nc.gpsimd.memset(fv[:, :, WP // 2 - 1:WP // 2 + 1], 0.0)
                nc.gpsimd.memset(fv[:, :, WP - 1:WP], 0.0)
            fv = v_b[:].bitcast(F32).rearrange("p (r c) -> p r c", c=WP2)
            nc.gpsimd.memset(fv[:, :, 0:C0], 0.0)
            nc.gpsimd.memset(fv[:, :, WP - C0:WP + C0], 0.0)
            nc.gpsimd.memset(fv[:, :, WP2 - C0:WP2], 0.0)

            # ---- projection: bf16 matmuls, contraction 256 via 2 psum passes,
            # ci-major so chunk 0 attention can start early ----
            pxc = [448, 448, 448, 448, 384]
            pxo = [0, 448, 896, 1344, 1792]

            def pad_view(tl, t, r0, nr, c0, cw=W):
                return _view(tl, r0 * WP2 + t * WP + c0, [(WP2, nr), (1, cw)])

            ps_stack = tc.tile_pool(name="psl", bufs=1, space="PSUM")
            psl = ps_stack.__enter__()
            pse_stack = tc.tile_pool(name="pse", bufs=1, space="PSUM")
            pse = pse_stack.__enter__()
            pso_stack = tc.tile_pool(name="pso", bufs=1, space="PSUM")
            pso = pso_stack.__enter__()

            def emit_proj(cis):
                for ci in cis:
                    cw, co = pxc[ci], pxo[ci]
                    r0, nr = co // W, cw // W
                    for m in range(6):
                        grp = m // 2        # 0=q, 1=k, 2=v
                        t = m % 2
                        dst = (q_b, k_b, v_b)[grp]
                        ppb = pse.tile([128, 2 * CHUNK], F32, tag="pe", name="pp", bufs=3)
                        pp = ppb[:, :cw]
                        for kt in range(2):
                            nc.tensor.matmul(
                                pp[:],
                                wt_t[kt][:, m * 128:(m + 1) * 128],
                                x_t[kt][:, co:co + cw],
                                start=(kt == 0),
                                stop=(kt == 1),
                            )
                        if grp == 2:
                            nc.vector.tensor_scalar_add(pad_view(dst, t, r0, nr, C0), pp[:],
                                                        bias_t[:, m:m + 1])
                        else:
                            nc.scalar.activation(pad_view(dst, t, r0, nr, C0), pp[:],
                                                 AF.Identity, bias=bias_t[:, m:m + 1])

            def emit_edge(row, col):
                # zero k/v halo row that falls outside the image
                for tl in (k_b, v_b):
                    rv = _view(tl, row * WP2, [(1, WP2)])
                    nc.gpsimd.tensor_scalar_mul(rv, rv, edge_t[:, col:col + 1])

            def gen_logits(ci, out):
                # pl [72,512] = pos-term + per-dl fp8 DoubleRow reductions
                pl = psl.tile([NL, CHUNK], F32, tag="pl", name="pl")
                for t in range(2):
                    nc.tensor.matmul(
                        pl[:], pos_r[t][:], pad_view(q_b, t, 1 + RC * ci, RC, C0),
                        start=(t == 0), stop=False, skip_group_check=True,
                    )
                qv = _view(q_b, (RC * ci + 1) * WP2 + C0,
                           [(WP, 2), (WP2, RC), (1, W)])
                for dl in range(NKK):
                    kv = _view(k_b, _off(ci, dl, 0), [(WP, 2), (WP2, RC), (1, W)])
                    pr = workp.tile([128, 2 * CHUNK], BF16, tag="pr", name=f"pr{dl}", bufs=6)
                    prw = pr[:].rearrange("p (i r c) -> p i r c", i=2, c=W)
                    eng = nc.vector if dl % 3 != 1 else nc.gpsimd
                    eng.tensor_mul(prw, qv, kv)
                    for t in range(2):
                        nc.tensor.matmul(
                            pl[:],
                            _view(red_r, (2 * dl + t) * NL, [(1, NL)]),
                            pr[:, t * CHUNK:(t + 1) * CHUNK],
                            start=False, stop=(dl == NKK - 1 and t == 1),
                            skip_group_check=True,
                        )
                    yield
                e_t = workp.tile([NL, CHUNK], F32R, tag="e", name="e", bufs=3)
                nc.scalar.activation(e_t[:], pl[:], AF.Exp)
                pz = pse.tile([NH, CHUNK], F32, tag="pzx", name="pz")
                nc.tensor.matmul(pz[:], sum9_r2[:], e_t[:],
                                 start=True, stop=True, skip_group_check=True)
                rz8 = workp.tile([NH, CHUNK], BF16, tag="rz8", name="rz8")
                with nc.allow_low_precision(reason="bf16 softmax denominators"):
                    nc.vector.reciprocal(rz8[:], pz[:])
                yield
                pzr = pse.tile([NL, CHUNK], F32, tag="pzx", name="pzr")
                nc.tensor.matmul(pzr[:], e8_r[:], rz8[:],
                                 start=True, stop=True, skip_group_check=True)
                attn = workp.tile([NL, CHUNK], F32R, tag="attn", name="attn", bufs=3)
                nc.vector.tensor_mul(attn[:], e_t[:], pzr[:].bitcast(F32R))
                out["attn"] = attn

            def gen_av(ci, attn):
                # t0 taps accumulate on PE into one psum bank; t1 taps
                # accumulate on Pool into an SBUF f32r accumulator (PE relief)
                po = pso.tile([128, CHUNK], F32, tag="po", name="po")
                po1 = acc1 = None
                if ci == NCHUNK - 1:
                    po1 = pso.tile([128, CHUNK], F32, tag="po1", name="po1")
                else:
                    acc1 = workp.tile([128, CHUNK], F32R, tag="acc1", name="acc1", bufs=2)
                pend = []

                def flush():
                    while pend:
                        pend.pop(0)()

                for j in range(4):
                    dl0 = 2 * j
                    for t in range(2):
                        pe = pse.tile([128, 2 * CHUNK], F32, tag="pe", name="pe", bufs=3)
                        for i in range(2):
                            nc.tensor.matmul(
                                pe[:, i * CHUNK:(i + 1) * CHUNK],
                                exp_r2[:, (dl0 + i) * CIN + t * 128:
                                      (dl0 + i) * CIN + (t + 1) * 128],
                                attn[:], start=True, stop=True, skip_group_check=True,
                            )
                        o0 = _off(ci, dl0, t)
                        vv = _view(v_b, o0, [(_off(ci, dl0 + 1, t) - o0, 2), (WP2, RC), (1, W)])
                        p2 = workp.tile([128, 2 * CHUNK], F32R, tag="p2", name=f"p2_{t}", bufs=6)
                        p2w = p2[:].rearrange("p (i r c) -> p i r c", i=2, c=W)
                        pew = pe[:].rearrange("p (i r c) -> p i r c", i=2, c=W)
                        if t == 0:
                            if j < 3:
                                # DVE reads psum directly (1x)
                                nc.vector.tensor_mul(p2w, pew.bitcast(F32R), vv)
                            else:
                                eb = workp.tile([128, 2 * CHUNK], F32R, tag="eb", name="eb0", bufs=3)
                                nc.scalar.copy(eb[:], pe[:])
                                ebw = eb[:].rearrange("p (i r c) -> p i r c", i=2, c=W)
                                nc.gpsimd.tensor_mul(p2w, ebw, vv)

                            def acc(p2=p2, j=j):
                                for i in range(2):
                                    nc.tensor.matmul(
                                        po[:], ident_r2[:],
                                        p2[:, i * CHUNK:(i + 1) * CHUNK],
                                        start=(j == 0 and i == 0), stop=False,
                                        skip_group_check=True,
                                    )
                            pend.append(acc)
                        else:
                            # ACT casts psum->f32r, Pool multiplies + accumulates
                            eb = workp.tile([128, 2 * CHUNK], F32R, tag="eb", name="eb1", bufs=3)
                            nc.scalar.copy(eb[:], pe[:])
                            ebw = eb[:].rearrange("p (i r c) -> p i r c", i=2, c=W)
                            nc.gpsimd.tensor_mul(p2w, ebw, vv)

                            if ci == NCHUNK - 1:
                                # drain: PE is idle, accumulate there instead
                                def acc(p2=p2, j=j):
                                    for i in range(2):
                                        nc.tensor.matmul(
                                            po1[:], ident_r2[:],
                                            p2[:, i * CHUNK:(i + 1) * CHUNK],
                                            start=(j == 0 and i == 0), stop=False,
                                            skip_group_check=True,
                                        )
                            else:
                                def acc(p2=p2, j=j):
                                    if j == 0:
                                        nc.gpsimd.tensor_add(acc1[:], p2[:, :CHUNK], p2[:, CHUNK:])
                                    else:
                                        nc.gpsimd.tensor_add(acc1[:], acc1[:], p2[:, :CHUNK])
                                        nc.gpsimd.tensor_add(acc1[:], acc1[:], p2[:, CHUNK:])
                            pend.append(acc)
                        yield
                        if len(pend) > 1:
                            pend.pop(0)()
                for t in range(2):
                    peb = pse.tile([128, 2 * CHUNK], F32, tag="pe", name="pes", bufs=3)
                    pe = peb[:, :CHUNK]
                    nc.tensor.matmul(
                        pe[:], exp_r2[:, 8 * CIN + t * 128: 8 * CIN + (t + 1) * 128],
                        attn[:], start=True, stop=True, skip_group_check=True,
                    )
                    vv = _view(v_b, _off(ci, 8, t), [(WP2, RC), (1, W)])
                    p2 = workp.tile([128, CHUNK], F32R, tag="p2s", name=f"p2s{t}", bufs=2)
                    p2w = p2[:].rearrange("p (r c) -> p r c", c=W)
                    pew = pe[:].rearrange("p (r c) -> p r c", c=W)
                    if t == 0:
                        nc.vector.tensor_mul(p2w, pew.bitcast(F32R), vv)

                        def accs(p2=p2):
                            nc.tensor.matmul(
                                po[:], ident_r2[:],
                                p2[:], start=False, stop=True, skip_group_check=True,
                            )
                        pend.append(accs)
                    else:
                        eb = workp.tile([128, CHUNK], F32R, tag="ebs", name="ebs", bufs=2)
                        nc.scalar.copy(eb[:], pe[:])
                        nc.gpsimd.tensor_mul(p2w, eb[:].rearrange("p (r c) -> p r c", c=W), vv)

                        if ci == NCHUNK - 1:
                            def accs(p2=p2):
                                nc.tensor.matmul(
                                    po1[:], ident_r2[:],
                                    p2[:], start=False, stop=True, skip_group_check=True,
                                )
                        else:
                            def accs(p2=p2):
                                nc.gpsimd.tensor_add(acc1[:], acc1[:], p2[:])
                        pend.append(accs)
                    yield
                    if len(pend) > 1:
                        pend.pop(0)()
                flush()
                ob = workp.tile([128, CHUNK], F32, tag="ob", name="ob", bufs=2)
                nc.scalar.copy(ob[:], po[:])
                nc.sync.dma_start(outd[0:128, ci * CHUNK:(ci + 1) * CHUNK], ob[:])
                if ci == NCHUNK - 1:
                    ob1 = workp.tile([128, CHUNK], F32, tag="ob1", name="ob1")
                    nc.vector.tensor_copy(ob1[:], po1[:])
                    nc.sync.dma_start(outd[128:256, ci * CHUNK:(ci + 1) * CHUNK], ob1[:])
                else:
                    nc.sync.dma_start(outd[128:256, ci * CHUNK:(ci + 1) * CHUNK],
                                      acc1[:].bitcast(F32))

            def gen_proj(cis):
                for ci in cis:
                    cw, co = pxc[ci], pxo[ci]
                    r0, nr = co // W, cw // W
                    for m in range(6):
                        grp = m // 2        # 0=q, 1=k, 2=v
                        t = m % 2
                        dst = (q_b, k_b, v_b)[grp]
                        ppb = pse.tile([128, 2 * CHUNK], F32, tag="pe", name="pp", bufs=3)
                        pp = ppb[:, :cw]
                        for kt in range(2):
                            nc.tensor.matmul(
                                pp[:],
                                wt_t[kt][:, m * 128:(m + 1) * 128],
                                x_t[kt][:, co:co + cw],
                                start=(kt == 0),
                                stop=(kt == 1),
                            )
                        if grp == 2 and t == 0:
                            nc.vector.tensor_scalar_add(pad_view(dst, t, r0, nr, C0), pp[:],
                                                        bias_t[:, m:m + 1])
                        else:
                            nc.scalar.activation(pad_view(dst, t, r0, nr, C0), pp[:],
                                                 AF.Identity, bias=bias_t[:, m:m + 1])
                        yield

            def zipper(*gens):
                live = [iter(g) for g in gens]
                while live:
                    nxt = []
                    for g in live:
                        try:
                            next(g)
                            nxt.append(g)
                        except StopIteration:
                            pass
                    live = nxt

            # software pipeline: zip logits(ci), AV(ci-1) and the next proj
            # group so every engine FIFO interleaves all concurrent stages
            hs = [{} for _ in range(NCHUNK)]
            zipper(gen_proj([0]))
            emit_edge(0, 0)
            zipper(gen_logits(0, hs[0]), gen_proj([1]))
            proj_at = {1: [2], 3: [3], 5: [4]}
            for ci in range(1, NCHUNK):
                gens = [gen_logits(ci, hs[ci]), gen_av(ci - 1, hs[ci - 1]["attn"])]
                if ci in proj_at:
                    gens.append(gen_proj(proj_at[ci]))
                if ci == 6:
                    emit_edge(HS - 1, 1)
                zipper(*gens)
            zipper(gen_av(NCHUNK - 1, hs[NCHUNK - 1]["attn"]))

            pso_stack.__exit__(None, None, None)
            pse_stack.__exit__(None, None, None)
            ps_stack.__exit__(None, None, None)

    nc.finalize()
    return nc


_CACHE = {}


def _host_consts(w_proj, b_proj, pos_feats):
    import ml_dtypes
    bf = ml_dtypes.bfloat16
    f8 = ml_dtypes.float8_e4m3

    wT = np.ascontiguousarray(w_proj.T).astype(np.float32).copy()   # [256, 768]
    wT[:, : 2 * QK] *= SCALE
    b = np.asarray(b_proj, np.float32).copy()
    b[: 2 * QK] *= SCALE
    bias = np.ascontiguousarray(b.reshape(6, 128).T)                # [128, 6]

    heads = np.arange(CIN) // D                                     # head of channel
    posblk = np.zeros((CIN, NL), np.float32)
    for c in range(CIN):
        n = heads[c]
        posblk[c, n * NKK:(n + 1) * NKK] = pos_feats[c]

    # redpair [128, 9*2*72] bf16: [k, dl, t, nl] = 1 iff nl == head(k+128t)*9+dl
    redpair = np.zeros((128, NKK, 2, NL), np.float32)
    for k in range(128):
        for t in range(2):
            n = heads[k + 128 * t]
            for dl in range(NKK):
                redpair[k, dl, t, n * NKK + dl] = 1.0

    sum9 = np.zeros((NL, NH), np.float32)
    e8 = np.zeros((NH, NL), np.float32)
    for n in range(NH):
        for dl in range(NKK):
            sum9[n * NKK + dl, n] = 1.0
            e8[n, n * NKK + dl] = 1.0

    expall = np.zeros((NL, NKK * CIN), np.float32)
    for dl in range(NKK):
        for n in range(NH):
            expall[n * NKK + dl, dl * CIN + n * D: dl * CIN + (n + 1) * D] = 1.0

    ident = np.eye(128, dtype=np.float32)

    return (wT.astype(np.float32), bias, posblk.astype(bf),
            redpair.reshape(128, NKK * 2 * NL).astype(bf),
            sum9.astype(np.float32), e8.astype(bf), expall.astype(np.float32),
            ident)


def make_in_maps(x, w_proj, b_proj, pos_feats):
    import ml_dtypes
    bf = ml_dtypes.bfloat16

    x = np.asarray(x, np.float32)
    wT, bias, posblk, redpair, sum9, e8, expall, identpair = _host_consts(
        np.asarray(w_proj, np.float32),
        np.asarray(b_proj, np.float32),
        np.asarray(pos_feats, np.float32),
    )
    in_maps = []
    for s in range(NCORES):
        b_i, half = s // 2, s % 2
        xs = np.zeros((CIN, HS, W), np.float32)
        h0 = half * HOWN - 1                  # global row of local row 0
        lo, hi = max(h0, 0), min(h0 + HS, H)
        xs[:, lo - h0:hi - h0, :] = x[b_i, :, lo:hi, :]
        edge = np.ones((128, 2), np.float32)
        if half == 0:
            edge[:, 0] = 0.0
        if half == 1:
            edge[:, 1] = 0.0
        in_maps.append({
            "x": np.ascontiguousarray(xs.reshape(CIN, PXU)),
            "wt": wT, "bias": bias, "posblk": posblk, "redpair": redpair,
            "sum9": sum9, "e8": e8, "expall": expall, "identpair": identpair,
            "edge": edge,
        })
    return in_maps


def kernel(x, w_proj, b_proj, pos_feats):
    from concourse.bass_utils import run_bass_kernel_spmd

    if "nc" not in _CACHE:
        _CACHE["nc"] = _build_nc()
    nc = _CACHE["nc"]
    in_maps = make_in_maps(x, w_proj, b_proj, pos_feats)
    res = run_bass_kernel_spmd(nc, in_maps, list(range(NCORES)))
    out = np.zeros((B, OUT, H, W), np.float32)
    for s in range(NCORES):
        b_i, half = s // 2, s % 2
        out[b_i, :, half * HOWN:(half + 1) * HOWN, :] = (
            res.results[s]["o"].reshape(OUT, HOWN, W)
        )
    return out
